# Initial kernel scaffold
#
"""Trainium2 Bass kernel for BatchedVectorAttention.

Reference (per batch element n, all shapes full):
    theta = x @ theta_w + theta_b          # [L, H]
    phi   = x @ phi_w + phi_b              # [L, H]
    psi   = x @ psi_w + psi_b              # [L, H]
    attn  = softmax(phi @ theta^T, axis=-1)    # [L, L]
    x_add = attn @ psi                     # [L, H]
    h1 = leaky_relu(x_add @ r1_w + r1_b, 0.2)
    h2 = tanh(h1 @ r2_w + r2_b)
    out = x + h2

Distribution: data-parallel over the batch dim n — one batch element per
NeuronCore (N=8 elements, 8 cores), identical SPMD program.

Per-core kernel strategy (all matmuls bf16 operands, fp32 PSUM accum):
  - x is supplied twice: natural fp32 [L, C] (residual add) and
    pre-transposed bf16 xT [C, L] (host-side layout prep).
  - Projections produce theta^T, phi^T as [H, L] (h on partitions) and psi
    as [L, H] (l on partitions) directly from matmuls — layouts chosen so
    the attention and MLP matmuls never need an on-device transpose.
  - Attention logits are computed TRANSPOSED: A^T[m, l] = sum_h
    thetaT[h, m] * phiT[h, l], so the softmax axis (m) lands on partitions
    of A^T. exp() runs on ScalarE (no max-subtraction: logits are O(10),
    fp32/bf16 exp cannot overflow), writing P^T bf16.
  - Row sums s[l] = sum_m P^T[m, l] via a ones-vector matmul (partition
    reduction on TensorE); normalization is DEFERRED: x_add^T is computed
    unnormalized as psi^T-blocks @ P^T, then multiplied by broadcast 1/s.
  - All five bias adds are folded into the matmul accumulation groups as
    K=1 rank-1 update matmuls (bias ⊗ ones).
  - MLP runs on the transposed activations: z^T = r1_w^T @ x_add^T,
    leaky-relu on ScalarE (Lrelu, alpha=0.2), then h2 = (h1^T)^T @ r2_w
    which lands back in NATURAL [l, c] layout for the residual + store.
"""

import os
from contextlib import ExitStack

import ml_dtypes
import numpy as np

N_CORES = 8
L_FULL = 2048
C = 512
H = 512
J = 256  # C // 2
SW = 512  # swath of l-columns processed per attention pass
P = 128

LAST_RESULTS = None
_BUILT = {}


def _build(L):
    import concourse.bass as bass  # noqa: F401
    import concourse.tile as tile
    from concourse import bacc, mybir

    bf16 = mybir.dt.bfloat16
    f32 = mybir.dt.float32
    AF = mybir.ActivationFunctionType

    CC = C // P  # 4 c-chunks
    HC = H // P  # 4 h-chunks
    JC = J // P  # 2 j-chunks
    NSW = L // SW  # swaths
    MB = L // P  # m-blocks (attention key dim)
    LB = SW // P  # l-blocks per swath

    nc = bacc.Bacc(
        "TRN2", target_bir_lowering=False, debug=False, enable_asserts=False
    )

    # weights and xT arrive pre-packed in the SBUF tile layout [P, k, n]
    # (host-side transpose) so each load is ONE fully-contiguous DMA.
    d_x = nc.dram_tensor("x", [L, C], f32, kind="ExternalInput")
    d_xT = nc.dram_tensor("xT", [P, C // P, L], bf16, kind="ExternalInput")
    d_thw = nc.dram_tensor("thw", [P, C // P, H], bf16, kind="ExternalInput")
    d_phw = nc.dram_tensor("phw", [P, C // P, H], bf16, kind="ExternalInput")
    d_psw = nc.dram_tensor("psw", [P, C // P, H], bf16, kind="ExternalInput")
    d_r1w = nc.dram_tensor("r1w", [P, H // P, J], bf16, kind="ExternalInput")
    d_r2w = nc.dram_tensor("r2w", [P, J // P, C], bf16, kind="ExternalInput")
    # per-partition bias columns ([128, n_chunks] fp32, host-pretransposed)
    d_thb = nc.dram_tensor("thb", [P, H // P], f32, kind="ExternalInput")
    d_phb = nc.dram_tensor("phb", [P, H // P], f32, kind="ExternalInput")
    d_r1b = nc.dram_tensor("r1b", [P, J // P], f32, kind="ExternalInput")
    # psi/r2 biases pre-broadcast to all partitions (host-side), folded into
    # the PSUM-drain ops on DVE (their bias axis is the free dim).
    d_psb = nc.dram_tensor("psb", [P, H], bf16, kind="ExternalInput")
    d_r2b = nc.dram_tensor("r2b", [P, C], bf16, kind="ExternalInput")
    d_out = nc.dram_tensor("out", [L, C], f32, kind="ExternalOutput")

    with tile.TileContext(nc) as tc, ExitStack() as ctx:
        const = ctx.enter_context(tc.tile_pool(name="const", bufs=1))
        big = ctx.enter_context(tc.tile_pool(name="big", bufs=1))
        ptp = ctx.enter_context(tc.tile_pool(name="ptp", bufs=2))
        work = ctx.enter_context(tc.tile_pool(name="work", bufs=2))
        io = ctx.enter_context(tc.tile_pool(name="io", bufs=3))
        # 8 PSUM banks total: pa 3 + ppv 3 + pmlp 2 (sums tile shares pmlp's
        # tag). 3 bufs on the matmul accumulators hides the ~100ns
        # slot-recycle semaphore latency otherwise paid by every 2nd group.
        pa = ctx.enter_context(tc.tile_pool(name="pa", bufs=3, space="PSUM"))
        ppv = ctx.enter_context(tc.tile_pool(name="ppv", bufs=3, space="PSUM"))
        pmlp = ctx.enter_context(tc.tile_pool(name="pmlp", bufs=2, space="PSUM"))

        # ---- PE warm-up first: ~7us of throwaway matmuls so the HAM
        # clock-gate opens (1.2 -> 2.4 GHz) while the DMA prologue runs.
        warm_in = const.tile([P, SW], bf16)
        nc.gpsimd.memset(warm_in, 0.0)
        for _ in range(12):
            wp = pa.tile([P, SW], f32, tag="acc", name="wp")
            nc.tensor.matmul(
                wp, lhsT=warm_in[:, 0:P], rhs=warm_in, start=True, stop=True
            )

        # ---- constants / inputs to SBUF ----
        # Two HWDGE queues exist (SP=nc.sync, Activation=nc.scalar): put the
        # first-needed tensors (theta/phi weights + xT) at the head of each so
        # the projections can start ~4us in, under the warm-up matmuls.
        def load_w(dram, k_chunks, n, eng):
            # pre-packed [P, k, n] in DRAM -> one fully-contiguous DMA
            t = const.tile([P, k_chunks, n], bf16, name=dram.name + "_t")
            eng.dma_start(out=t[:, :, :], in_=dram[:, :, :])
            return t

        # Prologue loads spread over three DMA paths (sync + scalar HWDGE
        # queues, gpsimd SWDGE) so theta_w + xT land right as the PE warm-up
        # ends; each queue backpressures at ~2-5us per 512KB transfer.
        thw_t = load_w(d_thw, CC, H, nc.sync)
        xT_t = big.tile([P, CC, L], bf16)
        for cc in range(CC):
            eng = nc.scalar if cc < CC // 2 else nc.sync
            eng.dma_start(out=xT_t[:, cc, :], in_=d_xT[:, cc, :])
        phw_t = load_w(d_phw, CC, H, nc.gpsimd)
        psw_t = load_w(d_psw, CC, H, nc.gpsimd)
        r1w_t = load_w(d_r1w, HC, J, nc.gpsimd)
        r2w_t = load_w(d_r2w, JC, C, nc.gpsimd)

        def load_b(dram, shape, dt):
            t = const.tile(shape, dt, name=dram.name + "_t")
            nc.sync.dma_start(out=t, in_=dram[:, :])
            return t

        thb_t = load_b(d_thb, [P, H // P], f32)
        phb_t = load_b(d_phb, [P, H // P], f32)
        r1b_t = load_b(d_r1b, [P, J // P], f32)
        psb_t = load_b(d_psb, [P, H], bf16)
        r2b_t = load_b(d_r2b, [P, C], bf16)

        # all-ones stationary matrix: out = ones^T @ x computes the
        # cross-partition column sums AND broadcasts them to all partitions.
        ones_mat = const.tile([P, P], bf16)
        nc.gpsimd.memset(ones_mat, 1.0)

        # ---- projections ----
        # thetaT/phiT: [h, l] = w^T @ x^T; bias folded as K=1 rank-1 matmul.
        thetaT_s = big.tile([P, HC, L], bf16)
        phiT_s = big.tile([P, HC, L], bf16)
        psi_s = big.tile([P, MB, H], bf16)

        for w_t, b_t, outT in ((thw_t, thb_t, thetaT_s), (phw_t, phb_t, phiT_s)):
            for sw in range(NSW):
                for hc in range(HC):
                    acc = pa.tile([P, SW], f32, tag="acc", name="acc")
                    for cc in range(CC):
                        nc.tensor.matmul(
                            acc,
                            lhsT=w_t[:, cc, hc * P : (hc + 1) * P],
                            rhs=xT_t[:, cc, sw * SW : (sw + 1) * SW],
                            start=(cc == 0),
                            stop=(cc == CC - 1),
                        )
                    # PSUM drain + per-partition (per-h) bias + bf16 cast.
                    # On ScalarE: ACT is idle during the projection phase,
                    # while DVE (psi drains) would otherwise stall the PE's
                    # PSUM slot recycling.
                    nc.scalar.activation(
                        out=outT[:, hc, sw * SW : (sw + 1) * SW],
                        in_=acc,
                        func=AF.Identity,
                        bias=b_t[:, hc : hc + 1],
                    )

        # psi: [l, h] = x @ psi_w (natural layout, l on partitions); the bias
        # rides along in the PSUM-drain add (psb pre-broadcast host-side).
        for mb in range(MB):
            acc2 = ppv.tile([P, H], f32, tag="pv", name="acc2")
            for cc in range(CC):
                nc.tensor.matmul(
                    acc2,
                    lhsT=xT_t[:, cc, mb * P : (mb + 1) * P],
                    rhs=psw_t[:, cc, :],
                    start=(cc == 0),
                    stop=(cc == CC - 1),
                )
            nc.vector.tensor_add(psi_s[:, mb, :], acc2, psb_t)

        # ---- attention + MLP, one swath of SW l-columns at a time ----
        for sw in range(NSW):
            lsl = slice(sw * SW, (sw + 1) * SW)

            # logits transposed + exp: P^T[m, l] = exp(A^T[m, l]).
            # Interleaved on DVE: a pairwise bf16 add-tree accumulates the
            # per-partition partial row sums of P^T across the 16 m-blocks.
            PT = ptp.tile([P, MB, SW], bf16, tag="PT", name="PT")
            tsum = work.tile([P, MB - 1, SW], bf16, tag="tsum", name="tsum")
            for mb in range(MB):
                at = pa.tile([P, SW], f32, tag="acc", name="at")
                for hc in range(HC):
                    nc.tensor.matmul(
                        at,
                        lhsT=thetaT_s[:, hc, mb * P : (mb + 1) * P],
                        rhs=phiT_s[:, hc, lsl],
                        start=(hc == 0),
                        stop=(hc == HC - 1),
                    )
                nc.scalar.activation(out=PT[:, mb, :], in_=at, func=AF.Exp)
                if mb % 2 == 1:
                    nc.vector.tensor_add(
                        tsum[:, mb // 2, :], PT[:, mb - 1, :], PT[:, mb, :]
                    )
            # remaining tree levels: slots [8..11], [12..13], [14]
            lo, n = 0, MB // 2
            while n > 1:
                for i in range(n // 2):
                    nc.vector.tensor_add(
                        tsum[:, lo + n + i, :],
                        tsum[:, lo + 2 * i, :],
                        tsum[:, lo + 2 * i + 1, :],
                    )
                lo, n = lo + n, n // 2

            # x_add^T[h, l] = sum_m psi[m, h] P^T[m, l], normalized by 1/s.
            # The ones-matrix matmul (cross-partition sum + broadcast in one
            # shot) is emitted after PV(hc=0) so the PE never waits on the
            # DVE add-tree tail.
            xaddT = work.tile([P, HC, SW], bf16, tag="xaddT", name="xaddT")
            rb = work.tile([P, SW], f32, tag="rb", name="rb")
            for hc in range(HC):
                pv = ppv.tile([P, SW], f32, tag="pv", name="pv")
                for mb in range(MB):
                    nc.tensor.matmul(
                        pv,
                        lhsT=psi_s[:, mb, hc * P : (hc + 1) * P],
                        rhs=PT[:, mb, :],
                        start=(mb == 0),
                        stop=(mb == MB - 1),
                    )
                if hc == 0:
                    st = pmlp.tile([P, SW], f32, tag="mlp", name="st")
                    nc.tensor.matmul(
                        st,
                        lhsT=ones_mat,
                        rhs=tsum[:, MB - 2, :],
                        start=True,
                        stop=True,
                    )
                    nc.vector.reciprocal_approx_fast(out=rb, in_=st)
                nc.vector.tensor_mul(out=xaddT[:, hc, :], in0=pv, in1=rb)

            # MLP layer 1 (transposed): z^T[j, l], leaky_relu(0.2) on ScalarE.
            h1T = work.tile([P, JC, SW], bf16, tag="h1T", name="h1T")
            for jc in range(JC):
                zt = pmlp.tile([P, SW], f32, tag="mlp", name="zt")
                for hc in range(HC):
                    nc.tensor.matmul(
                        zt,
                        lhsT=r1w_t[:, hc, jc * P : (jc + 1) * P],
                        rhs=xaddT[:, hc, :],
                        start=(hc == 0),
                        stop=(hc == HC - 1),
                    )
                # Prelu honors alpha (slope) exactly; Lrelu's slope is a
                # baked-in 0.01 regardless of alpha (HW-verified). bias (the
                # per-j r1_b column) is applied before the activation.
                nc.scalar.activation(
                    out=h1T[:, jc, :],
                    in_=zt,
                    func=AF.Prelu,
                    bias=r1b_t[:, jc : jc + 1],
                    alpha=0.2,
                )

            # MLP layer 2 back to natural layout + tanh + residual + store.
            # r2 bias rides the DVE PSUM-drain (in-place tanh on ACT after).
            for lb in range(LB):
                l0 = sw * SW + lb * P
                ht = pmlp.tile([P, C], f32, tag="mlp", name="ht")
                for jc in range(JC):
                    nc.tensor.matmul(
                        ht,
                        lhsT=h1T[:, jc, lb * P : (lb + 1) * P],
                        rhs=r2w_t[:, jc, :],
                        start=(jc == 0),
                        stop=(jc == JC - 1),
                    )
                h2 = io.tile([P, C], f32, tag="h2s", name="h2")
                nc.vector.tensor_add(h2, ht, r2b_t)
                nc.scalar.activation(out=h2, in_=h2, func=AF.Tanh)
                # gpsimd SWDGE: keeps these hoisted prefetches off the HWDGE
                # queues that feed the time-critical xT/weight prologue
                xn = io.tile([P, C], f32, tag="xn", name="xn")
                nc.gpsimd.dma_start(out=xn, in_=d_x[l0 : l0 + P, :])
                ot = io.tile([P, C], f32, tag="ot", name="ot")
                nc.vector.tensor_add(ot, h2, xn)
                nc.sync.dma_start(out=d_out[l0 : l0 + P, :], in_=ot)

    nc.compile()
    return nc


def _get_built(L):
    if L not in _BUILT:
        _BUILT[L] = _build(L)
    return _BUILT[L]


def _pack(w, n_out):
    # [K, n] -> SBUF tile layout [P, K//P, n], contiguous
    bf = ml_dtypes.bfloat16
    k = w.shape[0]
    return np.ascontiguousarray(
        w.reshape(k // P, P, n_out).transpose(1, 0, 2)
    ).astype(bf)


def _make_in_map(x_n, theta_w, theta_b, phi_w, phi_b, psi_w, psi_b, r1_w, r1_b, r2_w, r2_b):
    bf = ml_dtypes.bfloat16
    return {
        "x": np.ascontiguousarray(x_n, dtype=np.float32),
        "xT": _pack(np.ascontiguousarray(x_n.T), x_n.shape[0]),
        "thw": _pack(theta_w, H),
        "phw": _pack(phi_w, H),
        "psw": _pack(psi_w, H),
        "r1w": _pack(r1_w, J),
        "r2w": _pack(r2_w, C),
        "thb": np.ascontiguousarray(
            theta_b.reshape(H // P, P).T, dtype=np.float32
        ),
        "phb": np.ascontiguousarray(phi_b.reshape(H // P, P).T, dtype=np.float32),
        "r1b": np.ascontiguousarray(r1_b.reshape(J // P, P).T, dtype=np.float32),
        "psb": np.ascontiguousarray(
            np.broadcast_to(psi_b.reshape(1, H), (P, H))
        ).astype(bf),
        "r2b": np.ascontiguousarray(
            np.broadcast_to(r2_b.reshape(1, C), (P, C))
        ).astype(bf),
    }


def run(inputs: dict, n_cores: int = N_CORES, L: int = L_FULL):
    """Run the kernel on `n_cores` cores; batch element i goes to core i."""
    global LAST_RESULTS
    from concourse.bass_utils import run_bass_kernel_spmd

    nc = _get_built(L)
    x = np.asarray(inputs["x"], dtype=np.float32)
    assert x.shape == (n_cores, L, C), x.shape
    keys = (
        "theta_w", "theta_b", "phi_w", "phi_b", "psi_w", "psi_b",
        "r1_w", "r1_b", "r2_w", "r2_b",
    )
    ws = [np.asarray(inputs[k], dtype=np.float32) for k in keys]
    in_maps = [_make_in_map(x[n], *ws) for n in range(n_cores)]
    last_err = None
    for _ in range(3):
        try:
            res = run_bass_kernel_spmd(nc, in_maps, core_ids=list(range(n_cores)))
            break
        except Exception as e:  # transient NRT device wedge clears on retry
            last_err = e
    else:
        raise last_err
    LAST_RESULTS = res
    return np.stack([r["out"] for r in res.results])


def kernel(x, theta_w, theta_b, phi_w, phi_b, psi_w, psi_b, r1_w, r1_b, r2_w, r2_b):
    inputs = dict(
        x=x, theta_w=theta_w, theta_b=theta_b, phi_w=phi_w, phi_b=phi_b,
        psi_w=psi_w, psi_b=psi_b, r1_w=r1_w, r1_b=r1_b, r2_w=r2_w, r2_b=r2_b,
    )
    return run(inputs)


if __name__ == "__main__":
    os.environ.setdefault("JAX_PLATFORMS", "")
    rng = np.random.default_rng(0)
    Ltest = int(os.environ.get("KERNEL_TEST_L", "512"))
    ncores = int(os.environ.get("KERNEL_TEST_CORES", "1"))
    s = 0.02
    inputs = {
        "x": rng.standard_normal((ncores, Ltest, C), dtype=np.float32),
        "theta_w": rng.standard_normal((C, H), dtype=np.float32) * s,
        "theta_b": rng.standard_normal((H,), dtype=np.float32) * s,
        "phi_w": rng.standard_normal((C, H), dtype=np.float32) * s,
        "phi_b": rng.standard_normal((H,), dtype=np.float32) * s,
        "psi_w": rng.standard_normal((C, H), dtype=np.float32) * s,
        "psi_b": rng.standard_normal((H,), dtype=np.float32) * s,
        "r1_w": rng.standard_normal((H, J), dtype=np.float32) * s,
        "r1_b": rng.standard_normal((J,), dtype=np.float32) * s,
        "r2_w": rng.standard_normal((J, C), dtype=np.float32) * s,
        "r2_b": rng.standard_normal((C,), dtype=np.float32) * s,
    }
    actual = run(inputs, n_cores=ncores, L=Ltest)

    # numpy reference
    x = inputs["x"]
    outs = []
    for n in range(ncores):
        th = x[n] @ inputs["theta_w"] + inputs["theta_b"]
        ph = x[n] @ inputs["phi_w"] + inputs["phi_b"]
        psv = x[n] @ inputs["psi_w"] + inputs["psi_b"]
        a = ph @ th.T
        a = np.exp(a - a.max(axis=1, keepdims=True))
        attn = a / a.sum(axis=1, keepdims=True)
        xa = attn @ psv
        z = xa @ inputs["r1_w"] + inputs["r1_b"]
        h1 = np.where(z > 0, z, 0.2 * z)
        h2 = np.tanh(h1 @ inputs["r2_w"] + inputs["r2_b"])
        outs.append(x[n] + h2)
    expected = np.stack(outs)
    rel = np.linalg.norm(actual - expected) / np.linalg.norm(expected)
    print("small-test L2 rel err:", rel)
    print("max abs err:", np.abs(actual - expected).max())
    assert rel < 5e-3, rel
    print("SMALL TEST PASSED")



# revision 1
# speedup vs baseline: 1.0182x; 1.0182x over previous
"""Trainium2 Bass kernel for BatchedVectorAttention.

Reference (per batch element n, all shapes full):
    theta = x @ theta_w + theta_b          # [L, H]
    phi   = x @ phi_w + phi_b              # [L, H]
    psi   = x @ psi_w + psi_b              # [L, H]
    attn  = softmax(phi @ theta^T, axis=-1)    # [L, L]
    x_add = attn @ psi                     # [L, H]
    h1 = leaky_relu(x_add @ r1_w + r1_b, 0.2)
    h2 = tanh(h1 @ r2_w + r2_b)
    out = x + h2

Distribution: data-parallel over the batch dim n — one batch element per
NeuronCore (N=8 elements, 8 cores), identical SPMD program.

Per-core kernel strategy (all matmuls bf16 operands, fp32 PSUM accum):
  - x is supplied twice: natural fp32 [L, C] (residual add) and
    pre-transposed bf16 xT [C, L] (host-side layout prep).
  - Projections produce theta^T, phi^T as [H, L] (h on partitions) and psi
    as [L, H] (l on partitions) directly from matmuls — layouts chosen so
    the attention and MLP matmuls never need an on-device transpose.
  - Attention logits are computed TRANSPOSED: A^T[m, l] = sum_h
    thetaT[h, m] * phiT[h, l], so the softmax axis (m) lands on partitions
    of A^T. exp() runs on ScalarE (no max-subtraction: logits are O(10),
    fp32/bf16 exp cannot overflow), writing P^T bf16.
  - Row sums s[l] = sum_m P^T[m, l] via a ones-vector matmul (partition
    reduction on TensorE); normalization is DEFERRED: x_add^T is computed
    unnormalized as psi^T-blocks @ P^T, then multiplied by broadcast 1/s.
  - All five bias adds are folded into the matmul accumulation groups as
    K=1 rank-1 update matmuls (bias ⊗ ones).
  - MLP runs on the transposed activations: z^T = r1_w^T @ x_add^T,
    leaky-relu on ScalarE (Lrelu, alpha=0.2), then h2 = (h1^T)^T @ r2_w
    which lands back in NATURAL [l, c] layout for the residual + store.
"""

import os
from contextlib import ExitStack

import ml_dtypes
import numpy as np

N_CORES = 8
L_FULL = 2048
C = 512
H = 512
J = 256  # C // 2
SW = 512  # swath of l-columns processed per attention pass
P = 128

LAST_RESULTS = None
_BUILT = {}


def _build(L):
    import concourse.bass as bass  # noqa: F401
    import concourse.tile as tile
    from concourse import bacc, mybir

    bf16 = mybir.dt.bfloat16
    f32 = mybir.dt.float32
    AF = mybir.ActivationFunctionType

    CC = C // P  # 4 c-chunks
    HC = H // P  # 4 h-chunks
    JC = J // P  # 2 j-chunks
    NSW = L // SW  # swaths
    MB = L // P  # m-blocks (attention key dim)
    LB = SW // P  # l-blocks per swath

    nc = bacc.Bacc(
        "TRN2", target_bir_lowering=False, debug=False, enable_asserts=False
    )

    # weights and xT arrive pre-packed in the SBUF tile layout [P, k, n]
    # (host-side transpose) so each load is ONE fully-contiguous DMA.
    d_x = nc.dram_tensor("x", [L, C], f32, kind="ExternalInput")
    d_xT = nc.dram_tensor("xT", [P, C // P, L], bf16, kind="ExternalInput")
    d_thw = nc.dram_tensor("thw", [P, C // P, H], bf16, kind="ExternalInput")
    d_phw = nc.dram_tensor("phw", [P, C // P, H], bf16, kind="ExternalInput")
    d_psw = nc.dram_tensor("psw", [P, C // P, H], bf16, kind="ExternalInput")
    d_r1w = nc.dram_tensor("r1w", [P, H // P, J], bf16, kind="ExternalInput")
    d_r2w = nc.dram_tensor("r2w", [P, J // P, C], bf16, kind="ExternalInput")
    # per-partition bias columns ([128, n_chunks] fp32, host-pretransposed)
    d_thb = nc.dram_tensor("thb", [P, H // P], f32, kind="ExternalInput")
    d_phb = nc.dram_tensor("phb", [P, H // P], f32, kind="ExternalInput")
    d_r1b = nc.dram_tensor("r1b", [P, J // P], f32, kind="ExternalInput")
    # psi/r2 biases pre-broadcast to all partitions (host-side), folded into
    # the PSUM-drain ops on DVE (their bias axis is the free dim).
    d_psb = nc.dram_tensor("psb", [P, H], bf16, kind="ExternalInput")
    d_r2b = nc.dram_tensor("r2b", [P, C], bf16, kind="ExternalInput")
    d_out = nc.dram_tensor("out", [L, C], f32, kind="ExternalOutput")

    with tile.TileContext(nc) as tc, ExitStack() as ctx:
        const = ctx.enter_context(tc.tile_pool(name="const", bufs=1))
        big = ctx.enter_context(tc.tile_pool(name="big", bufs=1))
        ptp = ctx.enter_context(tc.tile_pool(name="ptp", bufs=2))
        work = ctx.enter_context(tc.tile_pool(name="work", bufs=2))
        io = ctx.enter_context(tc.tile_pool(name="io", bufs=3))
        # 8 PSUM banks total: pa 3 + ppv 3 + pmlp 2 (sums tile shares pmlp's
        # tag). 3 bufs on the matmul accumulators hides the ~100ns
        # slot-recycle semaphore latency otherwise paid by every 2nd group.
        pa = ctx.enter_context(tc.tile_pool(name="pa", bufs=3, space="PSUM"))
        ppv = ctx.enter_context(tc.tile_pool(name="ppv", bufs=3, space="PSUM"))
        pmlp = ctx.enter_context(tc.tile_pool(name="pmlp", bufs=2, space="PSUM"))

        # ---- PE warm-up first: ~7us of throwaway matmuls so the HAM
        # clock-gate opens (1.2 -> 2.4 GHz) while the DMA prologue runs.
        warm_in = const.tile([P, SW], bf16)
        nc.gpsimd.memset(warm_in, 0.0)
        for _ in range(12):
            wp = pa.tile([P, SW], f32, tag="acc", name="wp")
            nc.tensor.matmul(
                wp, lhsT=warm_in[:, 0:P], rhs=warm_in, start=True, stop=True
            )

        # ---- constants / inputs to SBUF ----
        # Two HWDGE queues exist (SP=nc.sync, Activation=nc.scalar): put the
        # first-needed tensors (theta/phi weights + xT) at the head of each so
        # the projections can start ~4us in, under the warm-up matmuls.
        def load_w(dram, k_chunks, n, eng):
            # pre-packed [P, k, n] in DRAM -> one fully-contiguous DMA
            t = const.tile([P, k_chunks, n], bf16, name=dram.name + "_t")
            eng.dma_start(out=t[:, :, :], in_=dram[:, :, :])
            return t

        # Prologue loads spread over three DMA paths (sync + scalar HWDGE
        # queues, gpsimd SWDGE) so theta_w + xT land right as the PE warm-up
        # ends; each queue backpressures at ~2-5us per 512KB transfer.
        thw_t = load_w(d_thw, CC, H, nc.sync)
        xT_t = big.tile([P, CC, L], bf16)
        for cc in range(CC):
            eng = nc.scalar if cc < CC // 2 else nc.sync
            eng.dma_start(out=xT_t[:, cc, :], in_=d_xT[:, cc, :])
        phw_t = load_w(d_phw, CC, H, nc.gpsimd)
        psw_t = load_w(d_psw, CC, H, nc.gpsimd)
        r1w_t = load_w(d_r1w, HC, J, nc.gpsimd)
        r2w_t = load_w(d_r2w, JC, C, nc.gpsimd)

        def load_b(dram, shape, dt):
            t = const.tile(shape, dt, name=dram.name + "_t")
            nc.sync.dma_start(out=t, in_=dram[:, :])
            return t

        thb_t = load_b(d_thb, [P, H // P], f32)
        phb_t = load_b(d_phb, [P, H // P], f32)
        r1b_t = load_b(d_r1b, [P, J // P], f32)
        psb_t = load_b(d_psb, [P, H], bf16)
        r2b_t = load_b(d_r2b, [P, C], bf16)

        # all-ones stationary matrix: out = ones^T @ x computes the
        # cross-partition column sums AND broadcasts them to all partitions.
        ones_mat = const.tile([P, P], bf16)
        nc.gpsimd.memset(ones_mat, 1.0)

        # ---- projections ----
        # thetaT/phiT: [h, l] = w^T @ x^T; bias folded as K=1 rank-1 matmul.
        thetaT_s = big.tile([P, HC, L], bf16)
        phiT_s = big.tile([P, HC, L], bf16)
        psi_s = big.tile([P, MB, H], bf16)

        for w_t, b_t, outT in ((thw_t, thb_t, thetaT_s), (phw_t, phb_t, phiT_s)):
            for sw in range(NSW):
                for hc in range(HC):
                    acc = pa.tile([P, SW], f32, tag="acc", name="acc")
                    for cc in range(CC):
                        nc.tensor.matmul(
                            acc,
                            lhsT=w_t[:, cc, hc * P : (hc + 1) * P],
                            rhs=xT_t[:, cc, sw * SW : (sw + 1) * SW],
                            start=(cc == 0),
                            stop=(cc == CC - 1),
                        )
                    # PSUM drain + per-partition (per-h) bias + bf16 cast.
                    # On ScalarE: ACT is idle during the projection phase,
                    # while DVE (psi drains) would otherwise stall the PE's
                    # PSUM slot recycling.
                    nc.scalar.activation(
                        out=outT[:, hc, sw * SW : (sw + 1) * SW],
                        in_=acc,
                        func=AF.Identity,
                        bias=b_t[:, hc : hc + 1],
                    )

        # psi: [l, h] = x @ psi_w (natural layout, l on partitions); the bias
        # rides along in the PSUM-drain add (psb pre-broadcast host-side).
        for mb in range(MB):
            acc2 = ppv.tile([P, H], f32, tag="pv", name="acc2")
            for cc in range(CC):
                nc.tensor.matmul(
                    acc2,
                    lhsT=xT_t[:, cc, mb * P : (mb + 1) * P],
                    rhs=psw_t[:, cc, :],
                    start=(cc == 0),
                    stop=(cc == CC - 1),
                )
            nc.vector.tensor_add(psi_s[:, mb, :], acc2, psb_t)

        # ---- attention + MLP, one swath of SW l-columns at a time ----
        for sw in range(NSW):
            lsl = slice(sw * SW, (sw + 1) * SW)

            # logits transposed + exp: P^T[m, l] = exp(A^T[m, l]).
            # Interleaved on DVE: a pairwise bf16 add-tree accumulates the
            # per-partition partial row sums of P^T across the 16 m-blocks.
            PT = ptp.tile([P, MB, SW], bf16, tag="PT", name="PT")
            tsum = work.tile([P, MB - 1, SW], bf16, tag="tsum", name="tsum")
            for mb in range(MB):
                at = pa.tile([P, SW], f32, tag="acc", name="at")
                for hc in range(HC):
                    nc.tensor.matmul(
                        at,
                        lhsT=thetaT_s[:, hc, mb * P : (mb + 1) * P],
                        rhs=phiT_s[:, hc, lsl],
                        start=(hc == 0),
                        stop=(hc == HC - 1),
                    )
                nc.scalar.activation(out=PT[:, mb, :], in_=at, func=AF.Exp)
                if mb % 2 == 1:
                    nc.vector.tensor_add(
                        tsum[:, mb // 2, :], PT[:, mb - 1, :], PT[:, mb, :]
                    )
            # remaining tree levels: slots [8..11], [12..13], [14]
            lo, n = 0, MB // 2
            while n > 1:
                for i in range(n // 2):
                    nc.vector.tensor_add(
                        tsum[:, lo + n + i, :],
                        tsum[:, lo + 2 * i, :],
                        tsum[:, lo + 2 * i + 1, :],
                    )
                lo, n = lo + n, n // 2

            # x_add^T[h, l] = sum_m psi[m, h] P^T[m, l], normalized by 1/s.
            # The ones-matrix matmul (cross-partition sum + broadcast in one
            # shot) is emitted after PV(hc=0) so the PE never waits on the
            # DVE add-tree tail.
            xaddT = work.tile([P, HC, SW], bf16, tag="xaddT", name="xaddT")
            rb = work.tile([P, SW], f32, tag="rb", name="rb")
            for hc in range(HC):
                pv = ppv.tile([P, SW], f32, tag="pv", name="pv")
                for mb in range(MB):
                    nc.tensor.matmul(
                        pv,
                        lhsT=psi_s[:, mb, hc * P : (hc + 1) * P],
                        rhs=PT[:, mb, :],
                        start=(mb == 0),
                        stop=(mb == MB - 1),
                    )
                if hc == 0:
                    st = pmlp.tile([P, SW], f32, tag="mlp", name="st")
                    nc.tensor.matmul(
                        st,
                        lhsT=ones_mat,
                        rhs=tsum[:, MB - 2, :],
                        start=True,
                        stop=True,
                    )
                    nc.vector.reciprocal_approx_fast(out=rb, in_=st)
                nc.vector.tensor_mul(out=xaddT[:, hc, :], in0=pv, in1=rb)

            # MLP layer 1 (transposed): z^T[j, l], leaky_relu(0.2) on ScalarE.
            h1T = work.tile([P, JC, SW], bf16, tag="h1T", name="h1T")
            for jc in range(JC):
                zt = pmlp.tile([P, SW], f32, tag="mlp", name="zt")
                for hc in range(HC):
                    nc.tensor.matmul(
                        zt,
                        lhsT=r1w_t[:, hc, jc * P : (jc + 1) * P],
                        rhs=xaddT[:, hc, :],
                        start=(hc == 0),
                        stop=(hc == HC - 1),
                    )
                # Prelu honors alpha (slope) exactly; Lrelu's slope is a
                # baked-in 0.01 regardless of alpha (HW-verified). bias (the
                # per-j r1_b column) is applied before the activation.
                nc.scalar.activation(
                    out=h1T[:, jc, :],
                    in_=zt,
                    func=AF.Prelu,
                    bias=r1b_t[:, jc : jc + 1],
                    alpha=0.2,
                )

            # MLP layer 2 back to natural layout + tanh + residual + store.
            # r2 bias rides the DVE PSUM-drain (in-place tanh on ACT after).
            for lb in range(LB):
                l0 = sw * SW + lb * P
                ht = pmlp.tile([P, C], f32, tag="mlp", name="ht")
                for jc in range(JC):
                    nc.tensor.matmul(
                        ht,
                        lhsT=h1T[:, jc, lb * P : (lb + 1) * P],
                        rhs=r2w_t[:, jc, :],
                        start=(jc == 0),
                        stop=(jc == JC - 1),
                    )
                h2 = io.tile([P, C], f32, tag="h2s", name="h2")
                nc.vector.tensor_add(h2, ht, r2b_t)
                nc.scalar.activation(out=h2, in_=h2, func=AF.Tanh)
                # gpsimd SWDGE: keeps these hoisted prefetches off the HWDGE
                # queues that feed the time-critical xT/weight prologue
                xn = io.tile([P, C], f32, tag="xn", name="xn")
                nc.gpsimd.dma_start(out=xn, in_=d_x[l0 : l0 + P, :])
                ot = io.tile([P, C], f32, tag="ot", name="ot")
                nc.vector.tensor_add(ot, h2, xn)
                nc.sync.dma_start(out=d_out[l0 : l0 + P, :], in_=ot)

    nc.compile()
    return nc


def _get_built(L):
    if L not in _BUILT:
        _BUILT[L] = _build(L)
    return _BUILT[L]


def _pack(w, n_out):
    # [K, n] -> SBUF tile layout [P, K//P, n], contiguous
    bf = ml_dtypes.bfloat16
    k = w.shape[0]
    return np.ascontiguousarray(
        w.reshape(k // P, P, n_out).transpose(1, 0, 2)
    ).astype(bf)


def _make_in_map(x_n, theta_w, theta_b, phi_w, phi_b, psi_w, psi_b, r1_w, r1_b, r2_w, r2_b):
    bf = ml_dtypes.bfloat16
    return {
        "x": np.ascontiguousarray(x_n, dtype=np.float32),
        "xT": _pack(np.ascontiguousarray(x_n.T), x_n.shape[0]),
        "thw": _pack(theta_w, H),
        "phw": _pack(phi_w, H),
        "psw": _pack(psi_w, H),
        "r1w": _pack(r1_w, J),
        "r2w": _pack(r2_w, C),
        "thb": np.ascontiguousarray(
            theta_b.reshape(H // P, P).T, dtype=np.float32
        ),
        "phb": np.ascontiguousarray(phi_b.reshape(H // P, P).T, dtype=np.float32),
        "r1b": np.ascontiguousarray(r1_b.reshape(J // P, P).T, dtype=np.float32),
        "psb": np.ascontiguousarray(
            np.broadcast_to(psi_b.reshape(1, H), (P, H))
        ).astype(bf),
        "r2b": np.ascontiguousarray(
            np.broadcast_to(r2_b.reshape(1, C), (P, C))
        ).astype(bf),
    }


def run(inputs: dict, n_cores: int = N_CORES, L: int = L_FULL):
    """Run the kernel on `n_cores` cores; batch element i goes to core i."""
    global LAST_RESULTS
    from concourse.bass_utils import run_bass_kernel_spmd

    nc = _get_built(L)
    x = np.asarray(inputs["x"], dtype=np.float32)
    assert x.shape == (n_cores, L, C), x.shape
    keys = (
        "theta_w", "theta_b", "phi_w", "phi_b", "psi_w", "psi_b",
        "r1_w", "r1_b", "r2_w", "r2_b",
    )
    ws = [np.asarray(inputs[k], dtype=np.float32) for k in keys]
    in_maps = [_make_in_map(x[n], *ws) for n in range(n_cores)]
    last_err = None
    for _ in range(3):
        try:
            res = run_bass_kernel_spmd(nc, in_maps, core_ids=list(range(n_cores)))
            break
        except Exception as e:  # transient NRT device wedge clears on retry
            last_err = e
    else:
        raise last_err
    LAST_RESULTS = res
    return np.stack([r["out"] for r in res.results])


def kernel(x, theta_w, theta_b, phi_w, phi_b, psi_w, psi_b, r1_w, r1_b, r2_w, r2_b):
    inputs = dict(
        x=x, theta_w=theta_w, theta_b=theta_b, phi_w=phi_w, phi_b=phi_b,
        psi_w=psi_w, psi_b=psi_b, r1_w=r1_w, r1_b=r1_b, r2_w=r2_w, r2_b=r2_b,
    )
    return run(inputs)


if __name__ == "__main__":
    os.environ.setdefault("JAX_PLATFORMS", "")
    rng = np.random.default_rng(0)
    Ltest = int(os.environ.get("KERNEL_TEST_L", "512"))
    ncores = int(os.environ.get("KERNEL_TEST_CORES", "1"))
    s = 0.02
    inputs = {
        "x": rng.standard_normal((ncores, Ltest, C), dtype=np.float32),
        "theta_w": rng.standard_normal((C, H), dtype=np.float32) * s,
        "theta_b": rng.standard_normal((H,), dtype=np.float32) * s,
        "phi_w": rng.standard_normal((C, H), dtype=np.float32) * s,
        "phi_b": rng.standard_normal((H,), dtype=np.float32) * s,
        "psi_w": rng.standard_normal((C, H), dtype=np.float32) * s,
        "psi_b": rng.standard_normal((H,), dtype=np.float32) * s,
        "r1_w": rng.standard_normal((H, J), dtype=np.float32) * s,
        "r1_b": rng.standard_normal((J,), dtype=np.float32) * s,
        "r2_w": rng.standard_normal((J, C), dtype=np.float32) * s,
        "r2_b": rng.standard_normal((C,), dtype=np.float32) * s,
    }
    actual = run(inputs, n_cores=ncores, L=Ltest)

    # numpy reference
    x = inputs["x"]
    outs = []
    for n in range(ncores):
        th = x[n] @ inputs["theta_w"] + inputs["theta_b"]
        ph = x[n] @ inputs["phi_w"] + inputs["phi_b"]
        psv = x[n] @ inputs["psi_w"] + inputs["psi_b"]
        a = ph @ th.T
        a = np.exp(a - a.max(axis=1, keepdims=True))
        attn = a / a.sum(axis=1, keepdims=True)
        xa = attn @ psv
        z = xa @ inputs["r1_w"] + inputs["r1_b"]
        h1 = np.where(z > 0, z, 0.2 * z)
        h2 = np.tanh(h1 @ inputs["r2_w"] + inputs["r2_b"])
        outs.append(x[n] + h2)
    expected = np.stack(outs)
    rel = np.linalg.norm(actual - expected) / np.linalg.norm(expected)
    print("small-test L2 rel err:", rel)
    print("max abs err:", np.abs(actual - expected).max())
    assert rel < 5e-3, rel
    print("SMALL TEST PASSED")

